# revision 67
# baseline (speedup 1.0000x reference)
"""Additive attention (B=4, Q=256, KV=1024, H=128, VS=256) on 8 Trainium2 cores.

Sharding: each core processes 32 query rows of every batch (4 groups of 32
row-slots).  Per batch, only a KV prefix of width ~valid_len (padded to even)
is computed; masked columns beyond it contribute exactly 0 to the softmax, so
skipping them is exact.  No collectives.  The program is specialized per
valid_lens configuration at call time and cached.

Per-core dataflow (ACT tanh is the hard floor: 1 elem/cycle/lane,
dtype-independent, ScalarE-only, ~49us/core for this config):
  PE  : q and k projections in fp16 (q-side fully fp16: Wq/qT rounding is
        far below the softmax noise floor)
  DVE : feats[h, kv] = fp16(kp16[h, kv] + qp32[h, s])  (tensor_scalar add)
  ACT : tanh in place over ramped row batches (the throughput floor); the
        first rows of group 0 are computed straight from DUPLICATE kp PSUM
        tiles via activation(bias=qp[:, s]) -- duplicates because the
        framework serializes a PSUM tile's DVE reads behind ACT's last
        read, which would stall the fp16 casts of the originals
  PE  : per-row one-hot fp16 matmuls with a [128, 32] wv window (cheap
        LDWEIGHTS, hidden under the previous matmul) accumulate score rows
        into the group's 32-partition band of a PSUM tile; the band is
        initialized by one K=2 matmul (band rows get the valid-len mask,
        non-band rows get MASK_VAL so they exp to exactly 0)
  per-group masked softmax: reduce_max (DVE) -> exp (ACT; column-strip
        split for the last group) -> probs "transposed" by probs.T @
        band-identity into a PSUM strip tile (PE) + row-sums via tiny
        pt.T @ ones matmuls into a spare pt_ps column (PE, keeps the DVE
        chain short) -> one fp16 cast (DVE) -> attn @ V in fp16 32-column
        bands (PE); the last group accumulates into the pad columns of its
        own score tile so it never serializes against the previous group's
        out reads.  Output scale: groups B-2/B-1 use the by-then-idle ACT
        engine (Copy activation with scale=1/rowsum) and issue their DMAs
        from the still-warm scalar queue; earlier groups scale on DVE and
        ship via sync/gpsimd.
Queue discipline: widest group first so its long tanh stream gives every
later group's adds enough runway; smallest group last for a short tail.
The head blob (Wk/Wq/qT32 + first 384 kT0 cols, fp16) is the first
sync-queue DMA; kt0 remainder + qT rest follow on sync; later kT chunks,
all constants and every V tile issue from the otherwise-idle gpsimd queue
(each dma_start costs ~600ns of issue time on its queue plus ~1.3us of
completion latency).  The static schedule is sim-driven, so explicit
cross-engine deps pin the hazards: kp casts after the first adds, the
previous group's pt cast after the tail adds, and its exp after the tail
tanhs.
"""
import math
import os
import sys

import numpy as np

for _p in ("/opt/trn_rl_repo", "/root/.axon_site/_ro/trn_rl_repo"):
    if os.path.isdir(_p):
        if _p not in sys.path:
            sys.path.insert(0, _p)
        break

B, Q, KV, QS, KS, H, VS = 4, 256, 1024, 128, 128, 128, 256
P = 128
N_CORES = 8
GROUP_ROWS = 32          # rows per (core, batch)
MASK_VAL = -30000.0      # large-negative that still fits fp16
HEAD_C1 = 384            # kt0 columns carried in the slim head blob

PROFILE = False          # set by test.py; enables NTFF tracing
LO_PASS = True           # kept for test.py compat (unused)
LAST_RESULTS = None
SIMULATE = False         # set by test.py; run CoreSim instead of hardware
LAST_EXEC_NS = None

_prog_cache = {}


def _build_program(cfg):
    """cfg: (Ws, l0flags): per-group computed KV widths in processing order
    and per-group valid_len==0 flags.  Returns nc."""
    Ws, l0flags = cfg
    import contextlib

    import concourse.bacc as bacc
    import concourse.mybir as mybir
    import concourse.tile as tile
    from concourse.tile_rust import add_dep_helper

    f32 = mybir.dt.float32
    f16 = mybir.dt.float16
    W = list(Ws)
    Wmax = max(W)
    SW = sum(W)
    offs = [sum(W[:i]) for i in range(B)]          # kp_sb column offsets
    nstrips = [(w + P - 1) // P for w in W]
    C1 = min(HEAD_C1, W[0])
    nc = bacc.Bacc("TRN2", target_bir_lowering=False, debug=False,
                   enable_asserts=True, num_devices=N_CORES)

    # head blob: wk16 | wq16 | qt32(f16) | kt0 cols 0:C1 -- everything the
    # first projections need in one DMA (a second DMA's completion latency
    # costs more than the extra bytes)
    HB = 256 + 256 + 64 + 2 * C1
    hblob_d = nc.dram_tensor("hblob", [P, HB], mybir.dt.uint8,
                             kind="ExternalInput").ap()
    blobq_d = nc.dram_tensor("blobq", [P, P - GROUP_ROWS], f16,
                             kind="ExternalInput").ap()
    kT_d = nc.dram_tensor("kT", [P, SW], f16, kind="ExternalInput").ap()
    V_d = nc.dram_tensor("V", [B, KV, VS], f16, kind="ExternalInput").ap()
    wvd_d = nc.dram_tensor("wvd", [P, 2 * GROUP_ROWS - 1], f16,
                           kind="ExternalInput").ap()
    ind_d = nc.dram_tensor("ind", [2, B * P], f16, kind="ExternalInput").ap()
    mask_d = nc.dram_tensor("mask", [2, B * Wmax], f16, kind="ExternalInput").ap()
    bident_d = nc.dram_tensor("bident", [P, GROUP_ROWS + 1], f16,
                              kind="ExternalInput").ap()
    out_d = nc.dram_tensor("out", [P, VS], f32, kind="ExternalOutput").ap()

    with tile.TileContext(nc) as tc, contextlib.ExitStack() as ctx:
        const = ctx.enter_context(tc.tile_pool(name="const", bufs=1))
        ktp = ctx.enter_context(tc.tile_pool(name="ktp", bufs=4))
        featsp = ctx.enter_context(tc.tile_pool(name="featsp", bufs=5))
        probsp = ctx.enter_context(tc.tile_pool(name="probsp", bufs=2))
        small = ctx.enter_context(tc.tile_pool(name="small", bufs=3))
        scp = ctx.enter_context(tc.tile_pool(name="scp", bufs=2, space="PSUM"))
        pmix = ctx.enter_context(tc.tile_pool(name="pmix", bufs=3, space="PSUM"))
        outp = ctx.enter_context(tc.tile_pool(name="outp", bufs=1, space="PSUM"))

        # ---- ACT table warm-up: load the exp/tanh spline set while the
        # first DMAs are still in flight ----
        warm = const.tile([1, 2], f16)
        nc.gpsimd.memset(warm[:], 0.0)
        nc.scalar.activation(warm[:], warm[:],
                             mybir.ActivationFunctionType.Tanh)

        # ---- head data: the slim blob first on sync, then kt0 remainder,
        # then qT rest; constants + V tiles go on the gpsimd queue ----
        hblob = const.tile([P, HB], mybir.dt.uint8)
        nc.sync.dma_start(hblob[:], hblob_d[:])
        wk_sb = hblob[:, 0:256].bitcast(f16)
        wq_sb = hblob[:, 256:512].bitcast(f16)
        qt32_sb = hblob[:, 512:576].bitcast(f16)

        # kt0 chunks: the first rides in the head blob; the rest are read
        # straight from kT (group 0 is at offset 0)
        kt0_chunks = [(0, C1, hblob[:, 576:HB].bitcast(f16))]
        for c0 in range(C1, W[0], 512):
            n = min(512, W[0] - c0)
            kt_t = ktp.tile([P, 512], f16, tag="kt", name=f"kt0_{c0}")
            nc.sync.dma_start(kt_t[:, :n], kT_d[:, c0:c0 + n])
            kt0_chunks.append((c0, n, kt_t))

        blobq = const.tile([P, P - GROUP_ROWS], f16)
        nc.sync.dma_start(blobq[:], blobq_d[:])

        wvd_t = const.tile([P, 2 * GROUP_ROWS - 1], f16)
        nc.gpsimd.dma_start(wvd_t[:], wvd_d[:])
        ind_sb = const.tile([2, B * P], f16)
        nc.gpsimd.dma_start(ind_sb[:], ind_d[:])
        mask_sb = const.tile([2, B * Wmax], f16)
        nc.gpsimd.dma_start(mask_sb[:], mask_d[:])
        bident = const.tile([P, GROUP_ROWS + 1], f16)
        nc.gpsimd.dma_start(bident[:], bident_d[:])

        kp_sb = const.tile([P, SW], f16)
        qp_sb = const.tile([P, P], f32)
        out_sb = const.tile([P, VS], f32)
        # one PSUM bank; groups alternate column halves so group i's attnV
        # never waits on group i-1's scale-out
        out_ps2 = outp.tile([P, 2 * VS], f32, name="out_ps2")
        rinv = small.tile([P, 1], f32, bufs=1, tag="rinv")
        vts = {}
        scores = [None] * B
        scpads = [0] * B
        nrms = [None] * B
        exp_instrs = [None] * B
        mulact_instrs = [None] * B

        kp_pss = {}

        def emit_load_mm(i, cs=512):
            """kT chunk DMAs + fp16 projections for group i.  The DMAs
            issue from the gpsimd queue: keeping the sync queue short lets
            the head blob's completion signal the first matmuls sooner."""
            w = W[i]
            kp_pss[i] = []
            for c0 in range(0, w, cs):
                n = min(cs, w - c0)
                kt_t = ktp.tile([P, 512], f16, tag="kt",
                                name=f"kt_{i}_{c0}")
                nc.gpsimd.dma_start(kt_t[:, :n],
                                    kT_d[:, offs[i] + c0:
                                         offs[i] + c0 + n])
                kp_ps = pmix.tile([P, 512], f32, tag="mix",
                                  name=f"kp_ps_{i}_{c0}")
                nc.tensor.matmul(kp_ps[:, :n], wk_sb, kt_t[:, :n],
                                 start=True, stop=True)
                kp_pss[i].append((c0, n, kp_ps))

        def emit_load_cast(i, after=None):
            """fp16 kp copies for group i (DVE side).  `after` pins them
            behind an earlier DVE instruction so the static schedule can't
            hoist them in front of adds they would stall."""
            insts = []
            for c0, n, kp_ps in kp_pss[i]:
                ci = nc.vector.tensor_copy(
                    kp_sb[:, offs[i] + c0: offs[i] + c0 + n],
                    kp_ps[:, :n])
                if after is not None:
                    add_dep_helper(ci.ins, after.ins,
                                   reason="kp cast after earlier adds")
                insts.append(ci)
            return insts

        def emit_mask(i):
            # K=2 rank-2 init: rows in the band get the valid-len mask, rows
            # outside it get MASK_VAL so they exp to exactly 0 later (the
            # probs->pt reduction matmul sums over all four bands).
            w = W[i]
            # pad to whole PSUM banks (512 f32) so 512-col matmul chunks
            # never cross a bank boundary
            wpad = ((w + 511) // 512) * 512
            sc = scp.tile([P, wpad], f32, tag="sc", name=f"scores_{i}")
            scores[i] = sc
            scpads[i] = wpad
            for c0 in range(0, w, 512):
                c1 = min(c0 + 512, w)
                nc.tensor.matmul(
                    sc[:, c0:c1],
                    ind_sb[0:2, i * P:(i + 1) * P],
                    mask_sb[0:2, i * Wmax + c0: i * Wmax + c1],
                    start=True, stop=l0flags[i] and c1 == w,
                    skip_group_check=True)

        def emit_score_mms(i, feats, r0, r1, c_lo, c_hi):
            """one-hot [128,32] wv-window matmuls accumulating rows
            r0..r1 of group i (cols c_lo:c_hi) into the group's band."""
            w = W[i]
            sc = scores[i]
            band = slice(GROUP_ROWS * i, GROUP_ROWS * (i + 1))
            for j in range(r1 - r0):
                s = r0 + j
                last_row = s == GROUP_ROWS - 1
                for c0 in range(c_lo, c_hi, 512):
                    c1 = min(c0 + 512, c_hi)
                    nc.tensor.matmul(
                        sc[band, c0:c1],
                        wvd_t[:, GROUP_ROWS - 1 - s: 2 * GROUP_ROWS - 1 - s],
                        feats[:, j * w + c0: j * w + c1],
                        start=False,
                        stop=last_row and c1 == c_hi,
                        tile_position=(0, GROUP_ROWS * i),
                        skip_group_check=True)

        def emit_scores(i, ranges):
            """adds + tanh + one-hot score matmuls for group i over the
            given (row0, row1) ranges.  Returns (adds, tanhs)."""
            w = W[i]
            adds = []
            tanhs = []
            for r0, r1 in ranges:
                nr = r1 - r0
                feats = featsp.tile([P, nr * w], f16, tag="feats",
                                    name=f"feats_{i}_{r0}")
                for j in range(nr):
                    s = GROUP_ROWS * i + r0 + j
                    adds.append(nc.vector.tensor_scalar_add(
                        feats[:, j * w:(j + 1) * w],
                        kp_sb[:, offs[i]: offs[i] + w],
                        qp_sb[:, s: s + 1]))
                tanhs.append(nc.scalar.activation(
                    feats[:], feats[:],
                    mybir.ActivationFunctionType.Tanh))
                emit_score_mms(i, feats, r0, r1, 0, w)
            return adds, tanhs

        def emit_vdma(i):
            for c in range(nstrips[i]):
                cw = min(P, W[i] - c * P)
                vts[(i, c)] = const.tile([P, VS], f16, name=f"v_{i}_{c}")
                nc.gpsimd.dma_start(vts[(i, c)][:cw, :],
                                    V_d[i, c * P: c * P + cw, :])

        def emit_rmax(i):
            # per-group -max; 0 outside the band so those rows (scores
            # MASK_VAL) exp to exactly 0.
            sc = scores[i]
            band = slice(GROUP_ROWS * i, GROUP_ROWS * (i + 1))
            nrm = small.tile([P, 1], f32, bufs=2, tag="nrm",
                             name=f"nrm_{i}")
            nrms[i] = nrm
            nc.vector.memset(nrm[:], 0.0)
            nc.vector.reduce_max(nrm[band, :], sc[band, 0:W[i]],
                                 axis=mybir.AxisListType.X, negate=True)

        def emit_softmax_attnv(i):
            w = W[i]
            n = nstrips[i]
            sc = scores[i]
            nrm = nrms[i]
            band = slice(GROUP_ROWS * i, GROUP_ROWS * (i + 1))
            wpad = n * P
            tail = i == B - 1
            probs = probsp.tile([P, wpad], f16, tag="probs",
                                name=f"probs_{i}")
            if wpad > w:
                nc.gpsimd.memset(probs[:, w:], 0.0)
            # one spare column holds the softmax denominators (pt.T @ ones
            # matmuls) -- same tile as pt so no new PSUM-tile serialization
            pt_ps = pmix.tile([P, GROUP_ROWS * n + 1], f32, tag="mix",
                              name=f"pt_ps_{i}")

            def emit_pt(c):
                # "transpose" probs via probs.T @ band-identity: full-height
                # stationary (base partition 0); non-band rows are exactly 0
                # so the cross-band sum picks out the band
                nc.tensor.matmul(pt_ps[:, GROUP_ROWS * c:
                                       GROUP_ROWS * (c + 1)],
                                 probs[:, c * P:(c + 1) * P],
                                 bident[:, 0:GROUP_ROWS],
                                 start=True, stop=True,
                                 skip_group_check=True)

            if tail and w > P:
                # strip-split exp so the first pt matmul overlaps the
                # second exp strip on the tail chain
                nc.scalar.activation(probs[:, 0:P], sc[:, 0:P],
                                     mybir.ActivationFunctionType.Exp,
                                     bias=nrm[:, 0:1], scale=1.0)
                emit_pt(0)
                exp_instrs[i] = nc.scalar.activation(
                    probs[:, P:w], sc[:, P:w],
                    mybir.ActivationFunctionType.Exp,
                    bias=nrm[:, 0:1], scale=1.0)
                for c in range(1, n):
                    emit_pt(c)
            else:
                exp_instrs[i] = nc.scalar.activation(
                    probs[:, :w], sc[:, 0:w],
                    mybir.ActivationFunctionType.Exp,
                    bias=nrm[:, 0:1], scale=1.0)
                for c in range(n):
                    emit_pt(c)
            pt_sb = small.tile([P, GROUP_ROWS * n], f16, tag="pt",
                               name=f"pt_sb_{i}")
            ptc = nc.vector.tensor_copy(pt_sb[:], pt_ps[:, 0:GROUP_ROWS * n])
            # the last group's attnV accumulates into the pad columns of its
            # own (tile-dep-wise already dead) score tile, so it never
            # serializes against the previous group's out_ps2 reads
            scpad = scpads[i]
            if tail and scpad - VS >= w:
                ot, base = scores[i], scpad - VS
            else:
                ot, base = out_ps2, (i % 2) * VS
            # softmax denominators: tiny pt.T @ ones matmuls (PE, off the
            # DVE chain)
            rs_col = GROUP_ROWS * n
            for c in range(n):
                cw = min(P, w - c * P)
                nc.tensor.matmul(
                    pt_ps[band, rs_col:rs_col + 1],
                    pt_sb[:cw, GROUP_ROWS * c: GROUP_ROWS * (c + 1)],
                    bident[:cw, GROUP_ROWS:GROUP_ROWS + 1],
                    start=(c == 0), stop=(c == n - 1),
                    tile_position=(0, GROUP_ROWS * i),
                    skip_group_check=True)
            nc.vector.reciprocal(rinv[band, :],
                                 pt_ps[band, rs_col:rs_col + 1])
            for c in range(n):
                cw = min(P, w - c * P)
                nc.tensor.matmul(
                    ot[band, base:base + VS],
                    pt_sb[:cw, GROUP_ROWS * c: GROUP_ROWS * (c + 1)],
                    vts[(i, c)][:cw, :],
                    start=(c == 0), stop=(c == n - 1),
                    tile_position=(0, GROUP_ROWS * i),
                    skip_group_check=True)
            # normalize + ship.  The last two groups run their scale on the
            # (by then idle) ACT engine via Copy's free affine -- the DVE
            # carries only the pt casts and reciprocals in the tail -- and
            # their DMAs issue from the still-warm scalar queue (sync and
            # gpsimd have been asleep for ~50us and pay a wake penalty
            # right on the retire path).
            if i >= B - 2:
                mulact_instrs[i] = nc.scalar.activation(
                    out_sb[band, :],
                    ot[band, base:base + VS],
                    mybir.ActivationFunctionType.Copy,
                    scale=rinv[band, 0:1])
                # only the LAST group's DMA issues from the scalar queue;
                # the next-to-last one would block the final Copy there
                q = nc.scalar if tail else nc.gpsimd
                q.dma_start(
                    out_d[GROUP_ROWS * i: GROUP_ROWS * (i + 1), :],
                    out_sb[band, :])
            else:
                nc.vector.tensor_scalar_mul(out_sb[band, :],
                                            ot[band, base:base + VS],
                                            rinv[band, 0:1])
                q = nc.sync if i % 2 == 0 else nc.gpsimd
                q.dma_start(out_d[GROUP_ROWS * i: GROUP_ROWS * (i + 1), :],
                            out_sb[band, :])
            return ptc

        # ---- head: project q rows 0:32 (slim blob), kp chunk 1, then the
        # first two rows' tanh straight from PSUM with bias=qp.  The head
        # tanhs read DUPLICATE kp projections: the framework serializes a
        # PSUM tile's DVE reads behind ACT's last read of the same tile, so
        # the fp16 casts get their own copies (PE is idle; the extra matmuls
        # are free) ----
        qp_ps = pmix.tile([P, GROUP_ROWS], f32, tag="mix", name="qp_ps")
        nc.tensor.matmul(qp_ps[:], wq_sb,
                         qt32_sb[:], start=True, stop=True,
                         skip_group_check=True)
        nc.vector.tensor_copy(qp_sb[:, 0:GROUP_ROWS], qp_ps[:])

        w0 = W[0]
        HEAD_ROWS = 3
        kp0_chunks = []
        dup_chunks = []
        if not l0flags[0]:
            # duplicate projections of the first two chunks: the head rows'
            # tanh reads these so the fp16 casts (reading the originals)
            # never serialize behind ACT
            c0, nn, kt_t = kt0_chunks[0]
            dup1 = pmix.tile([P, 512], f32, tag="mix", name="kp_dup_0")
            nc.tensor.matmul(dup1[:, :nn], wk_sb, kt_t[:, :nn],
                             start=True, stop=True)
            dup_chunks.append((c0, nn, dup1))
            if len(kt0_chunks) > 1:
                # the second dup chunk lives in the (still untouched) out
                # PSUM bank; attnV's start=True reset recycles it later
                c0, nn, kt_t = kt0_chunks[1]
                assert nn <= 2 * VS
                nc.tensor.matmul(out_ps2[:, 0:nn], wk_sb, kt_t[:, :nn],
                                 start=True, stop=True,
                                 skip_group_check=True)
                dup_chunks.append((c0, nn, out_ps2))
        for c0, nn, kt_t in kt0_chunks:
            kp_ps = pmix.tile([P, 512], f32, tag="mix",
                              name=f"kp_ps_0_{c0}")
            nc.tensor.matmul(kp_ps[:, :nn], wk_sb, kt_t[:, :nn],
                             start=True, stop=True)
            kp0_chunks.append((c0, nn, kp_ps))
        kp_pss[0] = kp0_chunks

        if not l0flags[0]:
            # first HEAD_ROWS rows of group 0: tanh(kp + qp[s]) via
            # activation bias, reading the duplicate kp PSUM chunks
            # (col-chunked so the first tanh starts as soon as data lands)
            feats0h = featsp.tile([P, HEAD_ROWS * w0], f16, tag="feats",
                                  name="feats_0_0")
            covered = sum(nn for _, nn, _ in dup_chunks)
            head_chunks = dup_chunks + [ch for ch in kp0_chunks
                                        if ch[0] >= covered]
            for j in range(HEAD_ROWS):
                for c0, nn, kp_ps in head_chunks:
                    nc.scalar.activation(
                        feats0h[:, j * w0 + c0: j * w0 + c0 + nn],
                        kp_ps[:, :nn],
                        mybir.ActivationFunctionType.Tanh,
                        bias=qp_sb[:, j: j + 1], scale=1.0)
            # kp casts for the rest of group 0's rows (from the originals,
            # which no ACT instruction ever reads)
            g0_casts = []
            for c0, nn, kp_ps in kp0_chunks:
                g0_casts.append(nc.vector.tensor_copy(
                    kp_sb[:, c0:c0 + nn], kp_ps[:, :nn]))
            emit_mask(0)
            emit_score_mms(0, feats0h, 0, HEAD_ROWS, 0, w0)
            adds48, _ = emit_scores(0, [(3, 5), (5, 9)])
            qp96_ps = pmix.tile([P, P - GROUP_ROWS], f32, tag="mix",
                                name="qp96_ps")
            nc.tensor.matmul(qp96_ps[:], wq_sb,
                             blobq[:], start=True, stop=True,
                             skip_group_check=True)
            qp96c = nc.vector.tensor_copy(qp_sb[:, GROUP_ROWS:],
                                          qp96_ps[:])
            # keep the head DVE chain in order: the g0 casts feed the first
            # adds; nothing may be scheduled in front of them
            add_dep_helper(qp96c.ins, g0_casts[-1].ins,
                           reason="qp rest copy after g0 kp casts")
            emit_load_mm(1)
            emit_scores(0, [(9, 17)])
            emit_load_mm(2)
            emit_load_mm(3)
            emit_scores(0, [(17, 32)])
        else:
            for c0, nn, kp_ps in kp0_chunks:
                nc.vector.tensor_copy(kp_sb[:, c0:c0 + nn], kp_ps[:, :nn])
            emit_mask(0)
            qp96_ps = pmix.tile([P, P - GROUP_ROWS], f32, tag="mix",
                                name="qp96_ps")
            nc.tensor.matmul(qp96_ps[:], wq_sb,
                             blobq[:], start=True, stop=True,
                             skip_group_check=True)
            nc.vector.tensor_copy(qp_sb[:, GROUP_ROWS:],
                                  qp96_ps[:])
            emit_load_mm(1)
            emit_load_mm(2)
            emit_load_mm(3)
            adds48 = None
        emit_load_cast(1, after=adds48[-1] if adds48 else None)
        emit_vdma(0)

        # ---- main loop: group i's first tanh batch precedes group i-1's
        # softmax; the last group's reduce_max precedes the exp-gated DVE
        # work of groups B-2/B-1 so the tail chain starts immediately ----
        for i in range(1, B):
            emit_mask(i)
            emit_vdma(i)
            if i + 1 < B:
                emit_load_cast(i + 1)
            if not l0flags[i]:
                if i < B - 1:
                    # split batches keep the PE score-matmul stream fed as
                    # soon as each half's tanh lands
                    emit_scores(i, [(0, 16)])
                    emit_rmax(i - 1)
                    emit_scores(i, [(16, 32)])
                    emit_softmax_attnv(i - 1)
                else:
                    emit_scores(i, [(0, 16)])
                    emit_rmax(i - 1)
                    # fine-grained last batches: the per-row score matmuls
                    # keep pace with the tanh stream and only the final
                    # row's matmul trails the last tanh
                    adds_t, tanhs_t = emit_scores(
                        i, [(16, 22), (22, 28), (28, 31), (31, 32)])
                    emit_rmax(i)
                    ptc_prev = emit_softmax_attnv(i - 1)
                    # the prev group's softmax work must never be scheduled
                    # in front of the tail group's last adds/tanh (DVE/ACT
                    # in-order streams would stall the tail on it)
                    add_dep_helper(ptc_prev.ins, adds_t[-1].ins,
                                   reason="prev pt cast after tail adds")
                    if exp_instrs[i - 1] is not None:
                        add_dep_helper(exp_instrs[i - 1].ins,
                                       tanhs_t[-1].ins,
                                       reason="prev exp after tail tanhs")
                    emit_softmax_attnv(i)
            else:
                emit_rmax(i - 1)
                emit_softmax_attnv(i - 1)
                if i == B - 1:
                    emit_rmax(i)
                    emit_softmax_attnv(i)
        # the next-to-last group's ACT-side out scale must never be
        # scheduled in front of the last group's exps on the ACT queue
        if mulact_instrs[B - 2] is not None and exp_instrs[B - 1] is not None:
            add_dep_helper(mulact_instrs[B - 2].ins,
                           exp_instrs[B - 1].ins,
                           reason="prev out scale after tail exps")

    nc.compile()
    return nc


def _get_program(cfg):
    if cfg not in _prog_cache:
        _prog_cache[cfg] = _build_program(cfg)
    return _prog_cache[cfg]


def _width(L):
    # even-padded computed width; valid_len==0 means "uniform over all KV"
    if L <= 0:
        return KV
    L = min(L, KV)
    return min(KV, max(2, 2 * math.ceil(L / 2)))


def kernel(queries, keys, values, valid_lens, Wq, Wk, wv):
    global LAST_EXEC_NS
    queries = np.ascontiguousarray(np.asarray(queries), dtype=np.float32)
    keys = np.ascontiguousarray(np.asarray(keys), dtype=np.float32)
    values = np.ascontiguousarray(np.asarray(values), dtype=np.float32)
    Wq = np.ascontiguousarray(np.asarray(Wq), dtype=np.float32)
    Wk = np.ascontiguousarray(np.asarray(Wk), dtype=np.float32)
    wv = np.ascontiguousarray(np.asarray(wv), dtype=np.float32)
    vl = [int(x) for x in np.asarray(valid_lens)]

    W_b = [_width(L) for L in vl]
    # widest group first: its long tanh stream gives the DVE adds of every
    # later group enough runway; smallest group last for a short tail
    gorder = sorted(range(B), key=lambda b: (-W_b[b], b))
    Ws = tuple(W_b[b] for b in gorder)
    l0flags = tuple(vl[b] == 0 for b in gorder)
    Wmax = max(Ws)

    nc = _get_program((Ws, l0flags))

    kT = np.concatenate(
        [keys[gorder[i]][:Ws[i]].T for i in range(B)], axis=1)
    kT = np.ascontiguousarray(kT.astype(np.float16))     # [128, SW]
    Vm = np.ascontiguousarray(
        np.stack([values[gorder[i]] for i in range(B)]).astype(np.float16))
    # row 0: band indicator x per-group valid mask; row 1: outside-band
    # indicator x MASK_VAL (so non-band score rows exp to exactly 0)
    ind = np.zeros((2, B * P), np.float16)
    for i in range(B):
        ind[0, i * P + GROUP_ROWS * i: i * P + GROUP_ROWS * (i + 1)] = 1.0
        ind[1, i * P: (i + 1) * P] = 1.0
        ind[1, i * P + GROUP_ROWS * i: i * P + GROUP_ROWS * (i + 1)] = 0.0
    mask = np.zeros((2, B * Wmax), np.float16)
    mask[1, :] = MASK_VAL
    for i in range(B):
        L = vl[gorder[i]]
        if L > 0:
            mask[0, i * Wmax + min(L, Ws[i]): i * Wmax + Ws[i]] = MASK_VAL
    # [128, 63] window: wv at col 31 so window [31-s : 63-s] puts wv at
    # in-band position s
    wvd = np.zeros((P, 2 * GROUP_ROWS - 1), np.float16)
    wvd[:, GROUP_ROWS - 1] = wv.astype(np.float16)
    # band identity + a trailing ones column (softmax denominator matmuls)
    bident = np.ascontiguousarray(np.concatenate(
        [np.tile(np.eye(GROUP_ROWS, dtype=np.float16), (B, 1)),
         np.ones((P, 1), np.float16)], axis=1))

    wk16u8 = np.ascontiguousarray(Wk.astype(np.float16)).view(np.uint8)
    wq16u8 = np.ascontiguousarray(Wq.astype(np.float16)).view(np.uint8)
    C1 = min(HEAD_C1, Ws[0])
    kt0c1u8 = np.ascontiguousarray(kT[:, 0:C1]).view(np.uint8)
    shared = {"kT": kT, "V": Vm, "ind": ind,
              "mask": mask, "wvd": wvd, "bident": bident}
    in_maps = []
    for c in range(N_CORES):
        qT = np.concatenate(
            [queries[gorder[i], c * GROUP_ROWS:(c + 1) * GROUP_ROWS, :].T
             for i in range(B)], axis=1).astype(np.float16)
        qt32u8 = np.ascontiguousarray(qT[:, 0:GROUP_ROWS]).view(np.uint8)
        m = dict(shared)
        m["blobq"] = np.ascontiguousarray(qT[:, GROUP_ROWS:])
        m["hblob"] = np.ascontiguousarray(
            np.concatenate([wk16u8, wq16u8, qt32u8, kt0c1u8], axis=1))
        in_maps.append(m)

    if SIMULATE:
        from concourse.bass_interp import CoreSim
        outs = []
        for c in range(N_CORES):
            sim = CoreSim(nc, trace=False)
            for name, v in in_maps[c].items():
                sim.tensor(name)[:] = v
            sim.simulate(check_with_hw=False)
            outs.append(sim.tensor("out").copy())
    else:
        from concourse import bass_utils
        kw = {}
        if PROFILE:
            kw = {"trace": True}
        res = bass_utils.run_bass_kernel_spmd(nc, in_maps, list(range(N_CORES)),
                                              **kw)
        if PROFILE:
            LAST_EXEC_NS = res.exec_time_ns
            global LAST_RESULTS
            LAST_RESULTS = res
        outs = [res.results[c]["out"] for c in range(N_CORES)]

    out = np.zeros((B, Q, VS), np.float32)
    for c in range(N_CORES):
        for i in range(B):
            out[gorder[i], c * GROUP_ROWS:(c + 1) * GROUP_ROWS, :] = \
                outs[c][GROUP_ROWS * i: GROUP_ROWS * (i + 1), :]
    return out


# revision 72
# speedup vs baseline: 1.0022x; 1.0022x over previous
"""Additive attention (B=4, Q=256, KV=1024, H=128, VS=256) on 8 Trainium2 cores.

Sharding: each core processes 32 query rows of every batch (4 groups of 32
row-slots).  Per batch, only a KV prefix of width ~valid_len (padded to even)
is computed; masked columns beyond it contribute exactly 0 to the softmax, so
skipping them is exact.  No collectives.  The program is specialized per
valid_lens configuration at call time and cached.

Per-core dataflow (ACT tanh is the hard floor: 1 elem/cycle/lane,
dtype-independent, ScalarE-only, ~49us/core for this config):
  PE  : q and k projections in fp16 (q-side fully fp16: Wq/qT rounding is
        far below the softmax noise floor)
  DVE : feats[h, kv] = fp16(kp16[h, kv] + qp32[h, s])  (tensor_scalar add)
  ACT : tanh in place over ramped row batches (the throughput floor); the
        first rows of group 0 are computed straight from DUPLICATE kp PSUM
        tiles via activation(bias=qp[:, s]) -- duplicates because the
        framework serializes a PSUM tile's DVE reads behind ACT's last
        read, which would stall the fp16 casts of the originals
  PE  : per-row one-hot fp16 matmuls with a [128, 32] wv window (cheap
        LDWEIGHTS, hidden under the previous matmul) accumulate score rows
        into the group's 32-partition band of a PSUM tile; the band is
        initialized by one K=2 matmul (band rows get the valid-len mask,
        non-band rows get MASK_VAL so they exp to exactly 0)
  per-group masked softmax: reduce_max (DVE) -> exp (ACT; column-strip
        split for the last group) -> probs "transposed" by probs.T @
        band-identity into a PSUM strip tile (PE) + row-sums via tiny
        pt.T @ ones matmuls into a spare pt_ps column (PE, keeps the DVE
        chain short) -> one fp16 cast (DVE) -> attn @ V in fp16 32-column
        bands (PE); the last group accumulates into the pad columns of its
        own score tile so it never serializes against the previous group's
        out reads.  Output scale: groups B-2/B-1 use the by-then-idle ACT
        engine (Copy activation with scale=1/rowsum) and issue their DMAs
        from the still-warm scalar queue; earlier groups scale on DVE and
        ship via sync/gpsimd.
Queue discipline: widest group first so its long tanh stream gives every
later group's adds enough runway; smallest group last for a short tail.
The head blob (Wk/Wq/qT32 + first 384 kT0 cols, fp16) is the first
sync-queue DMA; kt0 remainder + qT rest follow on sync; later kT chunks,
all constants and every V tile issue from the otherwise-idle gpsimd queue
(each dma_start costs ~600ns of issue time on its queue plus ~1.3us of
completion latency).  The static schedule is sim-driven, so explicit
cross-engine deps pin the hazards: kp casts after the first adds, the
previous group's pt cast after the tail adds, and its exp after the tail
tanhs.
"""
import math
import os
import sys

import numpy as np

for _p in ("/opt/trn_rl_repo", "/root/.axon_site/_ro/trn_rl_repo"):
    if os.path.isdir(_p):
        if _p not in sys.path:
            sys.path.insert(0, _p)
        break

B, Q, KV, QS, KS, H, VS = 4, 256, 1024, 128, 128, 128, 256
P = 128
N_CORES = 8
GROUP_ROWS = 32          # rows per (core, batch)
MASK_VAL = -30000.0      # large-negative that still fits fp16
HEAD_C1 = 384            # kt0 columns carried in the slim head blob

PROFILE = False          # set by test.py; enables NTFF tracing
LO_PASS = True           # kept for test.py compat (unused)
LAST_RESULTS = None
SIMULATE = False         # set by test.py; run CoreSim instead of hardware
LAST_EXEC_NS = None

_prog_cache = {}


def _build_program(cfg):
    """cfg: (Ws, l0flags): per-group computed KV widths in processing order
    and per-group valid_len==0 flags.  Returns nc."""
    Ws, l0flags = cfg
    import contextlib

    import concourse.bacc as bacc
    import concourse.mybir as mybir
    import concourse.tile as tile
    from concourse.tile_rust import add_dep_helper

    f32 = mybir.dt.float32
    f16 = mybir.dt.float16
    W = list(Ws)
    Wmax = max(W)
    SW = sum(W)
    offs = [sum(W[:i]) for i in range(B)]          # kp_sb column offsets
    nstrips = [(w + P - 1) // P for w in W]
    C1 = min(HEAD_C1, W[0])
    nc = bacc.Bacc("TRN2", target_bir_lowering=False, debug=False,
                   enable_asserts=True, num_devices=N_CORES)

    # head blob: wk16 | wq16 | qt32(f16) -- the first kt0 chunk issues in
    # parallel from the scalar queue, so the blob carries only the weights
    HB = 256 + 256 + 64
    hblob_d = nc.dram_tensor("hblob", [P, HB], mybir.dt.uint8,
                             kind="ExternalInput").ap()
    blobq_d = nc.dram_tensor("blobq", [P, P - GROUP_ROWS], f16,
                             kind="ExternalInput").ap()
    kT_d = nc.dram_tensor("kT", [P, SW], f16, kind="ExternalInput").ap()
    V_d = nc.dram_tensor("V", [B, KV, VS], f16, kind="ExternalInput").ap()
    wvd_d = nc.dram_tensor("wvd", [P, 2 * GROUP_ROWS - 1], f16,
                           kind="ExternalInput").ap()
    ind_d = nc.dram_tensor("ind", [2, B * P], f16, kind="ExternalInput").ap()
    mask_d = nc.dram_tensor("mask", [2, B * Wmax], f16, kind="ExternalInput").ap()
    bident_d = nc.dram_tensor("bident", [P, GROUP_ROWS + 1], f16,
                              kind="ExternalInput").ap()
    out_d = nc.dram_tensor("out", [P, VS], f32, kind="ExternalOutput").ap()

    with tile.TileContext(nc) as tc, contextlib.ExitStack() as ctx:
        const = ctx.enter_context(tc.tile_pool(name="const", bufs=1))
        ktp = ctx.enter_context(tc.tile_pool(name="ktp", bufs=4))
        featsp = ctx.enter_context(tc.tile_pool(name="featsp", bufs=5))
        probsp = ctx.enter_context(tc.tile_pool(name="probsp", bufs=2))
        small = ctx.enter_context(tc.tile_pool(name="small", bufs=3))
        scp = ctx.enter_context(tc.tile_pool(name="scp", bufs=2, space="PSUM"))
        pmix = ctx.enter_context(tc.tile_pool(name="pmix", bufs=3, space="PSUM"))
        outp = ctx.enter_context(tc.tile_pool(name="outp", bufs=1, space="PSUM"))

        # ---- the head-critical first kt0 chunk issues from the SCALAR
        # queue as its very first instruction: it transfers in parallel
        # with the sync-queue head blob, and the ACT table load slides in
        # right behind the ~0.7us issue ----
        kt0a_t = ktp.tile([P, 512], f16, tag="kt", name="kt0_0")
        nc.scalar.dma_start(kt0a_t[:, :C1], kT_d[:, 0:C1])

        # ---- ACT table warm-up: load the exp/tanh spline set while the
        # first DMAs are still in flight ----
        warm = const.tile([1, 2], f16)
        nc.gpsimd.memset(warm[:], 0.0)
        nc.scalar.activation(warm[:], warm[:],
                             mybir.ActivationFunctionType.Tanh)

        # ---- head data: the slim blob first on sync, then kt0 remainder,
        # then qT rest; constants + V tiles go on the gpsimd queue ----
        hblob = const.tile([P, HB], mybir.dt.uint8)
        nc.sync.dma_start(hblob[:], hblob_d[:])
        wk_sb = hblob[:, 0:256].bitcast(f16)
        wq_sb = hblob[:, 256:512].bitcast(f16)
        qt32_sb = hblob[:, 512:576].bitcast(f16)

        # kt0 chunks: the first was issued from the scalar queue above; the
        # rest are read straight from kT (group 0 is at offset 0)
        kt0_chunks = [(0, C1, kt0a_t)]
        for c0 in range(C1, W[0], 512):
            n = min(512, W[0] - c0)
            kt_t = ktp.tile([P, 512], f16, tag="kt", name=f"kt0_{c0}")
            nc.sync.dma_start(kt_t[:, :n], kT_d[:, c0:c0 + n])
            kt0_chunks.append((c0, n, kt_t))

        blobq = const.tile([P, P - GROUP_ROWS], f16)
        nc.sync.dma_start(blobq[:], blobq_d[:])

        wvd_t = const.tile([P, 2 * GROUP_ROWS - 1], f16)
        nc.gpsimd.dma_start(wvd_t[:], wvd_d[:])
        ind_sb = const.tile([2, B * P], f16)
        nc.gpsimd.dma_start(ind_sb[:], ind_d[:])
        mask_sb = const.tile([2, B * Wmax], f16)
        nc.gpsimd.dma_start(mask_sb[:], mask_d[:])
        bident = const.tile([P, GROUP_ROWS + 1], f16)
        nc.gpsimd.dma_start(bident[:], bident_d[:])

        kp_sb = const.tile([P, SW], f16)
        qp_sb = const.tile([P, P], f32)
        out_sb = const.tile([P, VS], f32)
        # one PSUM bank; groups alternate column halves so group i's attnV
        # never waits on group i-1's scale-out
        out_ps2 = outp.tile([P, 2 * VS], f32, name="out_ps2")
        rinv = small.tile([P, 1], f32, bufs=1, tag="rinv")
        vts = {}
        scores = [None] * B
        scpads = [0] * B
        nrms = [None] * B
        exp_instrs = [None] * B
        mulact_instrs = [None] * B

        kp_pss = {}

        def emit_load_mm(i, cs=512):
            """kT chunk DMAs + fp16 projections for group i.  The DMAs
            issue from the gpsimd queue: keeping the sync queue short lets
            the head blob's completion signal the first matmuls sooner."""
            w = W[i]
            kp_pss[i] = []
            for c0 in range(0, w, cs):
                n = min(cs, w - c0)
                kt_t = ktp.tile([P, 512], f16, tag="kt",
                                name=f"kt_{i}_{c0}")
                nc.gpsimd.dma_start(kt_t[:, :n],
                                    kT_d[:, offs[i] + c0:
                                         offs[i] + c0 + n])
                kp_ps = pmix.tile([P, 512], f32, tag="mix",
                                  name=f"kp_ps_{i}_{c0}")
                nc.tensor.matmul(kp_ps[:, :n], wk_sb, kt_t[:, :n],
                                 start=True, stop=True)
                kp_pss[i].append((c0, n, kp_ps))

        def emit_load_cast(i, after=None):
            """fp16 kp copies for group i (DVE side).  `after` pins them
            behind an earlier DVE instruction so the static schedule can't
            hoist them in front of adds they would stall."""
            insts = []
            for c0, n, kp_ps in kp_pss[i]:
                ci = nc.vector.tensor_copy(
                    kp_sb[:, offs[i] + c0: offs[i] + c0 + n],
                    kp_ps[:, :n])
                if after is not None:
                    add_dep_helper(ci.ins, after.ins,
                                   reason="kp cast after earlier adds")
                insts.append(ci)
            return insts

        def emit_mask(i):
            # K=2 rank-2 init: rows in the band get the valid-len mask, rows
            # outside it get MASK_VAL so they exp to exactly 0 later (the
            # probs->pt reduction matmul sums over all four bands).
            w = W[i]
            # pad to whole PSUM banks (512 f32) so 512-col matmul chunks
            # never cross a bank boundary
            wpad = ((w + 511) // 512) * 512
            sc = scp.tile([P, wpad], f32, tag="sc", name=f"scores_{i}")
            scores[i] = sc
            scpads[i] = wpad
            for c0 in range(0, w, 512):
                c1 = min(c0 + 512, w)
                nc.tensor.matmul(
                    sc[:, c0:c1],
                    ind_sb[0:2, i * P:(i + 1) * P],
                    mask_sb[0:2, i * Wmax + c0: i * Wmax + c1],
                    start=True, stop=l0flags[i] and c1 == w,
                    skip_group_check=True)

        def emit_score_mms(i, feats, r0, r1, c_lo, c_hi):
            """one-hot [128,32] wv-window matmuls accumulating rows
            r0..r1 of group i (cols c_lo:c_hi) into the group's band."""
            w = W[i]
            sc = scores[i]
            band = slice(GROUP_ROWS * i, GROUP_ROWS * (i + 1))
            for j in range(r1 - r0):
                s = r0 + j
                last_row = s == GROUP_ROWS - 1
                for c0 in range(c_lo, c_hi, 512):
                    c1 = min(c0 + 512, c_hi)
                    nc.tensor.matmul(
                        sc[band, c0:c1],
                        wvd_t[:, GROUP_ROWS - 1 - s: 2 * GROUP_ROWS - 1 - s],
                        feats[:, j * w + c0: j * w + c1],
                        start=False,
                        stop=last_row and c1 == c_hi,
                        tile_position=(0, GROUP_ROWS * i),
                        skip_group_check=True)

        def emit_scores(i, ranges):
            """adds + tanh + one-hot score matmuls for group i over the
            given (row0, row1) ranges.  Returns (adds, tanhs)."""
            w = W[i]
            adds = []
            tanhs = []
            for r0, r1 in ranges:
                nr = r1 - r0
                feats = featsp.tile([P, nr * w], f16, tag="feats",
                                    name=f"feats_{i}_{r0}")
                for j in range(nr):
                    s = GROUP_ROWS * i + r0 + j
                    adds.append(nc.vector.tensor_scalar_add(
                        feats[:, j * w:(j + 1) * w],
                        kp_sb[:, offs[i]: offs[i] + w],
                        qp_sb[:, s: s + 1]))
                tanhs.append(nc.scalar.activation(
                    feats[:], feats[:],
                    mybir.ActivationFunctionType.Tanh))
                emit_score_mms(i, feats, r0, r1, 0, w)
            return adds, tanhs

        def emit_vdma(i):
            for c in range(nstrips[i]):
                cw = min(P, W[i] - c * P)
                vts[(i, c)] = const.tile([P, VS], f16, name=f"v_{i}_{c}")
                nc.gpsimd.dma_start(vts[(i, c)][:cw, :],
                                    V_d[i, c * P: c * P + cw, :])

        def emit_rmax(i):
            # per-group -max; 0 outside the band so those rows (scores
            # MASK_VAL) exp to exactly 0.
            sc = scores[i]
            band = slice(GROUP_ROWS * i, GROUP_ROWS * (i + 1))
            nrm = small.tile([P, 1], f32, bufs=2, tag="nrm",
                             name=f"nrm_{i}")
            nrms[i] = nrm
            nc.vector.memset(nrm[:], 0.0)
            nc.vector.reduce_max(nrm[band, :], sc[band, 0:W[i]],
                                 axis=mybir.AxisListType.X, negate=True)

        def emit_softmax_attnv(i):
            w = W[i]
            n = nstrips[i]
            sc = scores[i]
            nrm = nrms[i]
            band = slice(GROUP_ROWS * i, GROUP_ROWS * (i + 1))
            wpad = n * P
            tail = i == B - 1
            probs = probsp.tile([P, wpad], f16, tag="probs",
                                name=f"probs_{i}")
            if wpad > w:
                nc.gpsimd.memset(probs[:, w:], 0.0)
            # one spare column holds the softmax denominators (pt.T @ ones
            # matmuls) -- same tile as pt so no new PSUM-tile serialization
            pt_ps = pmix.tile([P, GROUP_ROWS * n + 1], f32, tag="mix",
                              name=f"pt_ps_{i}")

            def emit_pt(c):
                # "transpose" probs via probs.T @ band-identity: full-height
                # stationary (base partition 0); non-band rows are exactly 0
                # so the cross-band sum picks out the band
                nc.tensor.matmul(pt_ps[:, GROUP_ROWS * c:
                                       GROUP_ROWS * (c + 1)],
                                 probs[:, c * P:(c + 1) * P],
                                 bident[:, 0:GROUP_ROWS],
                                 start=True, stop=True,
                                 skip_group_check=True)

            if tail and w > P:
                # strip-split exp so the first pt matmul overlaps the
                # second exp strip on the tail chain
                nc.scalar.activation(probs[:, 0:P], sc[:, 0:P],
                                     mybir.ActivationFunctionType.Exp,
                                     bias=nrm[:, 0:1], scale=1.0)
                emit_pt(0)
                exp_instrs[i] = nc.scalar.activation(
                    probs[:, P:w], sc[:, P:w],
                    mybir.ActivationFunctionType.Exp,
                    bias=nrm[:, 0:1], scale=1.0)
                for c in range(1, n):
                    emit_pt(c)
            else:
                exp_instrs[i] = nc.scalar.activation(
                    probs[:, :w], sc[:, 0:w],
                    mybir.ActivationFunctionType.Exp,
                    bias=nrm[:, 0:1], scale=1.0)
                for c in range(n):
                    emit_pt(c)
            pt_sb = small.tile([P, GROUP_ROWS * n], f16, tag="pt",
                               name=f"pt_sb_{i}")
            ptc = nc.vector.tensor_copy(pt_sb[:], pt_ps[:, 0:GROUP_ROWS * n])
            # the last group's attnV accumulates into the pad columns of its
            # own (tile-dep-wise already dead) score tile, so it never
            # serializes against the previous group's out_ps2 reads
            scpad = scpads[i]
            if tail and scpad - VS >= w:
                ot, base = scores[i], scpad - VS
            else:
                ot, base = out_ps2, (i % 2) * VS
            # softmax denominators: tiny pt.T @ ones matmuls (PE, off the
            # DVE chain)
            rs_col = GROUP_ROWS * n
            for c in range(n):
                cw = min(P, w - c * P)
                nc.tensor.matmul(
                    pt_ps[band, rs_col:rs_col + 1],
                    pt_sb[:cw, GROUP_ROWS * c: GROUP_ROWS * (c + 1)],
                    bident[:cw, GROUP_ROWS:GROUP_ROWS + 1],
                    start=(c == 0), stop=(c == n - 1),
                    tile_position=(0, GROUP_ROWS * i),
                    skip_group_check=True)
            nc.vector.reciprocal(rinv[band, :],
                                 pt_ps[band, rs_col:rs_col + 1])
            for c in range(n):
                cw = min(P, w - c * P)
                nc.tensor.matmul(
                    ot[band, base:base + VS],
                    pt_sb[:cw, GROUP_ROWS * c: GROUP_ROWS * (c + 1)],
                    vts[(i, c)][:cw, :],
                    start=(c == 0), stop=(c == n - 1),
                    tile_position=(0, GROUP_ROWS * i),
                    skip_group_check=True)
            # normalize + ship.  The last two groups run their scale on the
            # (by then idle) ACT engine via Copy's free affine -- the DVE
            # carries only the pt casts and reciprocals in the tail -- and
            # their DMAs issue from the still-warm scalar queue (sync and
            # gpsimd have been asleep for ~50us and pay a wake penalty
            # right on the retire path).
            if i >= B - 2:
                mulact_instrs[i] = nc.scalar.activation(
                    out_sb[band, :],
                    ot[band, base:base + VS],
                    mybir.ActivationFunctionType.Copy,
                    scale=rinv[band, 0:1])
                # only the LAST group's DMA issues from the scalar queue;
                # the next-to-last one would block the final Copy there
                q = nc.scalar if tail else nc.gpsimd
                q.dma_start(
                    out_d[GROUP_ROWS * i: GROUP_ROWS * (i + 1), :],
                    out_sb[band, :])
            else:
                nc.vector.tensor_scalar_mul(out_sb[band, :],
                                            ot[band, base:base + VS],
                                            rinv[band, 0:1])
                q = nc.sync if i % 2 == 0 else nc.gpsimd
                q.dma_start(out_d[GROUP_ROWS * i: GROUP_ROWS * (i + 1), :],
                            out_sb[band, :])
            return ptc

        # ---- head: project q rows 0:32 (slim blob), kp chunk 1, then the
        # first two rows' tanh straight from PSUM with bias=qp.  The head
        # tanhs read DUPLICATE kp projections: the framework serializes a
        # PSUM tile's DVE reads behind ACT's last read of the same tile, so
        # the fp16 casts get their own copies (PE is idle; the extra matmuls
        # are free) ----
        qp_ps = pmix.tile([P, GROUP_ROWS], f32, tag="mix", name="qp_ps")
        nc.tensor.matmul(qp_ps[:], wq_sb,
                         qt32_sb[:], start=True, stop=True,
                         skip_group_check=True)
        nc.vector.tensor_copy(qp_sb[:, 0:GROUP_ROWS], qp_ps[:])

        w0 = W[0]
        HEAD_ROWS = 3
        kp0_chunks = []
        dup_chunks = []
        if not l0flags[0]:
            # duplicate projections of the first two chunks: the head rows'
            # tanh reads these so the fp16 casts (reading the originals)
            # never serialize behind ACT
            c0, nn, kt_t = kt0_chunks[0]
            dup1 = pmix.tile([P, 512], f32, tag="mix", name="kp_dup_0")
            nc.tensor.matmul(dup1[:, :nn], wk_sb, kt_t[:, :nn],
                             start=True, stop=True)
            dup_chunks.append((c0, nn, dup1))
            if len(kt0_chunks) > 1:
                # the second dup chunk lives in the (still untouched) out
                # PSUM bank; attnV's start=True reset recycles it later
                c0, nn, kt_t = kt0_chunks[1]
                assert nn <= 2 * VS
                nc.tensor.matmul(out_ps2[:, 0:nn], wk_sb, kt_t[:, :nn],
                                 start=True, stop=True,
                                 skip_group_check=True)
                dup_chunks.append((c0, nn, out_ps2))
        for c0, nn, kt_t in kt0_chunks:
            kp_ps = pmix.tile([P, 512], f32, tag="mix",
                              name=f"kp_ps_0_{c0}")
            nc.tensor.matmul(kp_ps[:, :nn], wk_sb, kt_t[:, :nn],
                             start=True, stop=True)
            kp0_chunks.append((c0, nn, kp_ps))
        kp_pss[0] = kp0_chunks

        if not l0flags[0]:
            # first HEAD_ROWS rows of group 0: tanh(kp + qp[s]) via
            # activation bias, reading the duplicate kp PSUM chunks
            # (col-chunked so the first tanh starts as soon as data lands)
            feats0h = featsp.tile([P, HEAD_ROWS * w0], f16, tag="feats",
                                  name="feats_0_0")
            covered = sum(nn for _, nn, _ in dup_chunks)
            head_chunks = dup_chunks + [ch for ch in kp0_chunks
                                        if ch[0] >= covered]
            for j in range(HEAD_ROWS):
                for c0, nn, kp_ps in head_chunks:
                    nc.scalar.activation(
                        feats0h[:, j * w0 + c0: j * w0 + c0 + nn],
                        kp_ps[:, :nn],
                        mybir.ActivationFunctionType.Tanh,
                        bias=qp_sb[:, j: j + 1], scale=1.0)
            # kp casts for the rest of group 0's rows (from the originals,
            # which no ACT instruction ever reads)
            g0_casts = []
            for c0, nn, kp_ps in kp0_chunks:
                g0_casts.append(nc.vector.tensor_copy(
                    kp_sb[:, c0:c0 + nn], kp_ps[:, :nn]))
            emit_mask(0)
            emit_score_mms(0, feats0h, 0, HEAD_ROWS, 0, w0)
            adds48, _ = emit_scores(0, [(3, 5), (5, 9)])
            qp96_ps = pmix.tile([P, P - GROUP_ROWS], f32, tag="mix",
                                name="qp96_ps")
            nc.tensor.matmul(qp96_ps[:], wq_sb,
                             blobq[:], start=True, stop=True,
                             skip_group_check=True)
            qp96c = nc.vector.tensor_copy(qp_sb[:, GROUP_ROWS:],
                                          qp96_ps[:])
            # keep the head DVE chain in order: the g0 casts feed the first
            # adds; nothing may be scheduled in front of them
            add_dep_helper(qp96c.ins, g0_casts[-1].ins,
                           reason="qp rest copy after g0 kp casts")
            emit_load_mm(1)
            emit_scores(0, [(9, 17)])
            emit_load_mm(2)
            emit_load_mm(3)
            emit_scores(0, [(17, 32)])
        else:
            for c0, nn, kp_ps in kp0_chunks:
                nc.vector.tensor_copy(kp_sb[:, c0:c0 + nn], kp_ps[:, :nn])
            emit_mask(0)
            qp96_ps = pmix.tile([P, P - GROUP_ROWS], f32, tag="mix",
                                name="qp96_ps")
            nc.tensor.matmul(qp96_ps[:], wq_sb,
                             blobq[:], start=True, stop=True,
                             skip_group_check=True)
            nc.vector.tensor_copy(qp_sb[:, GROUP_ROWS:],
                                  qp96_ps[:])
            emit_load_mm(1)
            emit_load_mm(2)
            emit_load_mm(3)
            adds48 = None
        emit_load_cast(1, after=adds48[-1] if adds48 else None)
        emit_vdma(0)

        # ---- main loop: group i's first tanh batch precedes group i-1's
        # softmax; the last group's reduce_max precedes the exp-gated DVE
        # work of groups B-2/B-1 so the tail chain starts immediately ----
        for i in range(1, B):
            emit_mask(i)
            emit_vdma(i)
            if i + 1 < B:
                emit_load_cast(i + 1)
            if not l0flags[i]:
                if i < B - 1:
                    # split batches keep the PE score-matmul stream fed as
                    # soon as each half's tanh lands
                    emit_scores(i, [(0, 16)])
                    emit_rmax(i - 1)
                    emit_scores(i, [(16, 32)])
                    emit_softmax_attnv(i - 1)
                else:
                    emit_scores(i, [(0, 16)])
                    emit_rmax(i - 1)
                    # fine-grained last batches: the per-row score matmuls
                    # keep pace with the tanh stream and only the final
                    # row's matmul trails the last tanh
                    adds_t, tanhs_t = emit_scores(
                        i, [(16, 22), (22, 28), (28, 31), (31, 32)])
                    emit_rmax(i)
                    ptc_prev = emit_softmax_attnv(i - 1)
                    # the prev group's softmax work must never be scheduled
                    # in front of the tail group's last adds/tanh (DVE/ACT
                    # in-order streams would stall the tail on it)
                    add_dep_helper(ptc_prev.ins, adds_t[-1].ins,
                                   reason="prev pt cast after tail adds")
                    if exp_instrs[i - 1] is not None:
                        add_dep_helper(exp_instrs[i - 1].ins,
                                       tanhs_t[-1].ins,
                                       reason="prev exp after tail tanhs")
                    emit_softmax_attnv(i)
            else:
                emit_rmax(i - 1)
                emit_softmax_attnv(i - 1)
                if i == B - 1:
                    emit_rmax(i)
                    emit_softmax_attnv(i)
        # the next-to-last group's ACT-side out scale must never be
        # scheduled in front of the last group's exps on the ACT queue
        if mulact_instrs[B - 2] is not None and exp_instrs[B - 1] is not None:
            add_dep_helper(mulact_instrs[B - 2].ins,
                           exp_instrs[B - 1].ins,
                           reason="prev out scale after tail exps")

    nc.compile()
    return nc


def _get_program(cfg):
    if cfg not in _prog_cache:
        _prog_cache[cfg] = _build_program(cfg)
    return _prog_cache[cfg]


def _width(L):
    # even-padded computed width; valid_len==0 means "uniform over all KV"
    if L <= 0:
        return KV
    L = min(L, KV)
    return min(KV, max(2, 2 * math.ceil(L / 2)))


def kernel(queries, keys, values, valid_lens, Wq, Wk, wv):
    global LAST_EXEC_NS
    queries = np.ascontiguousarray(np.asarray(queries), dtype=np.float32)
    keys = np.ascontiguousarray(np.asarray(keys), dtype=np.float32)
    values = np.ascontiguousarray(np.asarray(values), dtype=np.float32)
    Wq = np.ascontiguousarray(np.asarray(Wq), dtype=np.float32)
    Wk = np.ascontiguousarray(np.asarray(Wk), dtype=np.float32)
    wv = np.ascontiguousarray(np.asarray(wv), dtype=np.float32)
    vl = [int(x) for x in np.asarray(valid_lens)]

    W_b = [_width(L) for L in vl]
    # widest group first: its long tanh stream gives the DVE adds of every
    # later group enough runway; smallest group last for a short tail
    gorder = sorted(range(B), key=lambda b: (-W_b[b], b))
    Ws = tuple(W_b[b] for b in gorder)
    l0flags = tuple(vl[b] == 0 for b in gorder)
    Wmax = max(Ws)

    nc = _get_program((Ws, l0flags))

    kT = np.concatenate(
        [keys[gorder[i]][:Ws[i]].T for i in range(B)], axis=1)
    kT = np.ascontiguousarray(kT.astype(np.float16))     # [128, SW]
    Vm = np.ascontiguousarray(
        np.stack([values[gorder[i]] for i in range(B)]).astype(np.float16))
    # row 0: band indicator x per-group valid mask; row 1: outside-band
    # indicator x MASK_VAL (so non-band score rows exp to exactly 0)
    ind = np.zeros((2, B * P), np.float16)
    for i in range(B):
        ind[0, i * P + GROUP_ROWS * i: i * P + GROUP_ROWS * (i + 1)] = 1.0
        ind[1, i * P: (i + 1) * P] = 1.0
        ind[1, i * P + GROUP_ROWS * i: i * P + GROUP_ROWS * (i + 1)] = 0.0
    mask = np.zeros((2, B * Wmax), np.float16)
    mask[1, :] = MASK_VAL
    for i in range(B):
        L = vl[gorder[i]]
        if L > 0:
            mask[0, i * Wmax + min(L, Ws[i]): i * Wmax + Ws[i]] = MASK_VAL
    # [128, 63] window: wv at col 31 so window [31-s : 63-s] puts wv at
    # in-band position s
    wvd = np.zeros((P, 2 * GROUP_ROWS - 1), np.float16)
    wvd[:, GROUP_ROWS - 1] = wv.astype(np.float16)
    # band identity + a trailing ones column (softmax denominator matmuls)
    bident = np.ascontiguousarray(np.concatenate(
        [np.tile(np.eye(GROUP_ROWS, dtype=np.float16), (B, 1)),
         np.ones((P, 1), np.float16)], axis=1))

    wk16u8 = np.ascontiguousarray(Wk.astype(np.float16)).view(np.uint8)
    wq16u8 = np.ascontiguousarray(Wq.astype(np.float16)).view(np.uint8)
    shared = {"kT": kT, "V": Vm, "ind": ind,
              "mask": mask, "wvd": wvd, "bident": bident}
    in_maps = []
    for c in range(N_CORES):
        qT = np.concatenate(
            [queries[gorder[i], c * GROUP_ROWS:(c + 1) * GROUP_ROWS, :].T
             for i in range(B)], axis=1).astype(np.float16)
        qt32u8 = np.ascontiguousarray(qT[:, 0:GROUP_ROWS]).view(np.uint8)
        m = dict(shared)
        m["blobq"] = np.ascontiguousarray(qT[:, GROUP_ROWS:])
        m["hblob"] = np.ascontiguousarray(
            np.concatenate([wk16u8, wq16u8, qt32u8], axis=1))
        in_maps.append(m)

    if SIMULATE:
        from concourse.bass_interp import CoreSim
        outs = []
        for c in range(N_CORES):
            sim = CoreSim(nc, trace=False)
            for name, v in in_maps[c].items():
                sim.tensor(name)[:] = v
            sim.simulate(check_with_hw=False)
            outs.append(sim.tensor("out").copy())
    else:
        from concourse import bass_utils
        kw = {}
        if PROFILE:
            kw = {"trace": True}
        res = bass_utils.run_bass_kernel_spmd(nc, in_maps, list(range(N_CORES)),
                                              **kw)
        if PROFILE:
            LAST_EXEC_NS = res.exec_time_ns
            global LAST_RESULTS
            LAST_RESULTS = res
        outs = [res.results[c]["out"] for c in range(N_CORES)]

    out = np.zeros((B, Q, VS), np.float32)
    for c in range(N_CORES):
        for i in range(B):
            out[gorder[i], c * GROUP_ROWS:(c + 1) * GROUP_ROWS, :] = \
                outs[c][GROUP_ROWS * i: GROUP_ROWS * (i + 1), :]
    return out


# revision 74
# speedup vs baseline: 1.0054x; 1.0032x over previous
"""Additive attention (B=4, Q=256, KV=1024, H=128, VS=256) on 8 Trainium2 cores.

Sharding: each core processes 32 query rows of every batch (4 groups of 32
row-slots).  Per batch, only a KV prefix of width ~valid_len (padded to even)
is computed; masked columns beyond it contribute exactly 0 to the softmax, so
skipping them is exact.  No collectives.  The program is specialized per
valid_lens configuration at call time and cached.

Per-core dataflow (ACT tanh is the hard floor: 1 elem/cycle/lane,
dtype-independent, ScalarE-only, ~49us/core for this config):
  PE  : q and k projections in fp16 (q-side fully fp16: Wq/qT rounding is
        far below the softmax noise floor)
  DVE : feats[h, kv] = fp16(kp16[h, kv] + qp32[h, s])  (tensor_scalar add)
  ACT : tanh in place over ramped row batches (the throughput floor); the
        first rows of group 0 are computed straight from DUPLICATE kp PSUM
        tiles via activation(bias=qp[:, s]) -- duplicates because the
        framework serializes a PSUM tile's DVE reads behind ACT's last
        read, which would stall the fp16 casts of the originals
  PE  : per-row one-hot fp16 matmuls with a [128, 32] wv window (cheap
        LDWEIGHTS, hidden under the previous matmul) accumulate score rows
        into the group's 32-partition band of a PSUM tile; the band is
        initialized by one K=2 matmul (band rows get the valid-len mask,
        non-band rows get MASK_VAL so they exp to exactly 0)
  per-group masked softmax: reduce_max (DVE) -> exp (ACT; column-strip
        split for the last group) -> probs "transposed" by probs.T @
        band-identity into a PSUM strip tile (PE) + row-sums via tiny
        pt.T @ ones matmuls into a spare pt_ps column (PE, keeps the DVE
        chain short) -> one fp16 cast (DVE) -> attn @ V in fp16 32-column
        bands (PE); the last group accumulates into the pad columns of its
        own score tile so it never serializes against the previous group's
        out reads.  Output scale: groups B-2/B-1 use the by-then-idle ACT
        engine (Copy activation with scale=1/rowsum) and issue their DMAs
        from the still-warm scalar queue; earlier groups scale on DVE and
        ship via sync/gpsimd.
Queue discipline: widest group first so its long tanh stream gives every
later group's adds enough runway; smallest group last for a short tail.
The head blob (Wk/Wq/qT32 + first 384 kT0 cols, fp16) is the first
sync-queue DMA; kt0 remainder + qT rest follow on sync; later kT chunks,
all constants and every V tile issue from the otherwise-idle gpsimd queue
(each dma_start costs ~600ns of issue time on its queue plus ~1.3us of
completion latency).  The static schedule is sim-driven, so explicit
cross-engine deps pin the hazards: kp casts after the first adds, the
previous group's pt cast after the tail adds, and its exp after the tail
tanhs.
"""
import math
import os
import sys

import numpy as np

for _p in ("/opt/trn_rl_repo", "/root/.axon_site/_ro/trn_rl_repo"):
    if os.path.isdir(_p):
        if _p not in sys.path:
            sys.path.insert(0, _p)
        break

B, Q, KV, QS, KS, H, VS = 4, 256, 1024, 128, 128, 128, 256
P = 128
N_CORES = 8
GROUP_ROWS = 32          # rows per (core, batch)
MASK_VAL = -30000.0      # large-negative that still fits fp16
HEAD_C1 = 384            # kt0 columns carried in the slim head blob

PROFILE = False          # set by test.py; enables NTFF tracing
LO_PASS = True           # kept for test.py compat (unused)
LAST_RESULTS = None
SIMULATE = False         # set by test.py; run CoreSim instead of hardware
LAST_EXEC_NS = None

_prog_cache = {}


def _build_program(cfg):
    """cfg: (Ws, l0flags): per-group computed KV widths in processing order
    and per-group valid_len==0 flags.  Returns nc."""
    Ws, l0flags = cfg
    import contextlib

    import concourse.bacc as bacc
    import concourse.mybir as mybir
    import concourse.tile as tile
    from concourse.tile_rust import add_dep_helper

    f32 = mybir.dt.float32
    f16 = mybir.dt.float16
    W = list(Ws)
    Wmax = max(W)
    SW = sum(W)
    offs = [sum(W[:i]) for i in range(B)]          # kp_sb column offsets
    nstrips = [(w + P - 1) // P for w in W]
    C1 = min(HEAD_C1, W[0])
    nc = bacc.Bacc("TRN2", target_bir_lowering=False, debug=False,
                   enable_asserts=True, num_devices=N_CORES)

    # head blob: wk16 | wq16 | qt32(f16) -- the first kt0 chunk issues in
    # parallel from the scalar queue, so the blob carries only the weights
    HB = 256 + 256 + 64
    hblob_d = nc.dram_tensor("hblob", [P, HB], mybir.dt.uint8,
                             kind="ExternalInput").ap()
    blobq_d = nc.dram_tensor("blobq", [P, P - GROUP_ROWS], f16,
                             kind="ExternalInput").ap()
    kT_d = nc.dram_tensor("kT", [P, SW], f16, kind="ExternalInput").ap()
    V_d = nc.dram_tensor("V", [B, KV, VS], f16, kind="ExternalInput").ap()
    wvd_d = nc.dram_tensor("wvd", [P, 2 * GROUP_ROWS - 1], f16,
                           kind="ExternalInput").ap()
    ind_d = nc.dram_tensor("ind", [2, B * P], f16, kind="ExternalInput").ap()
    mask_d = nc.dram_tensor("mask", [2, B * Wmax], f16, kind="ExternalInput").ap()
    bident_d = nc.dram_tensor("bident", [P, GROUP_ROWS + 1], f16,
                              kind="ExternalInput").ap()
    out_d = nc.dram_tensor("out", [P, VS], f32, kind="ExternalOutput").ap()

    with tile.TileContext(nc) as tc, contextlib.ExitStack() as ctx:
        const = ctx.enter_context(tc.tile_pool(name="const", bufs=1))
        ktp = ctx.enter_context(tc.tile_pool(name="ktp", bufs=4))
        featsp = ctx.enter_context(tc.tile_pool(name="featsp", bufs=5))
        probsp = ctx.enter_context(tc.tile_pool(name="probsp", bufs=2))
        small = ctx.enter_context(tc.tile_pool(name="small", bufs=3))
        scp = ctx.enter_context(tc.tile_pool(name="scp", bufs=2, space="PSUM"))
        pmix = ctx.enter_context(tc.tile_pool(name="pmix", bufs=3, space="PSUM"))
        outp = ctx.enter_context(tc.tile_pool(name="outp", bufs=1, space="PSUM"))

        # ---- the head-critical kt0 chunks issue from the SCALAR queue as
        # its very first instructions: the sequencer runs them concurrently
        # with the ACT table load, and they transfer in parallel with the
        # sync-queue head blob ----
        kt0_chunks = []
        c0 = 0
        while c0 < W[0]:
            n = min(C1 if c0 == 0 else 512, W[0] - c0)
            kt_t = ktp.tile([P, 512], f16, tag="kt", name=f"kt0_{c0}")
            nc.scalar.dma_start(kt_t[:, :n], kT_d[:, c0:c0 + n])
            kt0_chunks.append((c0, n, kt_t))
            c0 += n

        # ---- ACT table warm-up: load the exp/tanh spline set while the
        # first DMAs are still in flight ----
        warm = const.tile([1, 2], f16)
        nc.gpsimd.memset(warm[:], 0.0)
        nc.scalar.activation(warm[:], warm[:],
                             mybir.ActivationFunctionType.Tanh)

        # ---- head data: the slim blob first on sync, then kt0 remainder,
        # then qT rest; constants + V tiles go on the gpsimd queue ----
        hblob = const.tile([P, HB], mybir.dt.uint8)
        nc.sync.dma_start(hblob[:], hblob_d[:])
        wk_sb = hblob[:, 0:256].bitcast(f16)
        wq_sb = hblob[:, 256:512].bitcast(f16)
        qt32_sb = hblob[:, 512:576].bitcast(f16)

        blobq = const.tile([P, P - GROUP_ROWS], f16)
        nc.sync.dma_start(blobq[:], blobq_d[:])

        wvd_t = const.tile([P, 2 * GROUP_ROWS - 1], f16)
        nc.gpsimd.dma_start(wvd_t[:], wvd_d[:])
        ind_sb = const.tile([2, B * P], f16)
        nc.gpsimd.dma_start(ind_sb[:], ind_d[:])
        mask_sb = const.tile([2, B * Wmax], f16)
        nc.gpsimd.dma_start(mask_sb[:], mask_d[:])
        bident = const.tile([P, GROUP_ROWS + 1], f16)
        nc.gpsimd.dma_start(bident[:], bident_d[:])

        kp_sb = const.tile([P, SW], f16)
        qp_sb = const.tile([P, P], f32)
        out_sb = const.tile([P, VS], f32)
        # one PSUM bank; groups alternate column halves so group i's attnV
        # never waits on group i-1's scale-out
        out_ps2 = outp.tile([P, 2 * VS], f32, name="out_ps2")
        rinv = small.tile([P, 1], f32, bufs=1, tag="rinv")
        vts = {}
        scores = [None] * B
        scpads = [0] * B
        nrms = [None] * B
        exp_instrs = [None] * B
        mulact_instrs = [None] * B

        kp_pss = {}

        def emit_load_mm(i, cs=512):
            """kT chunk DMAs + fp16 projections for group i.  The DMAs
            issue from the gpsimd queue: keeping the sync queue short lets
            the head blob's completion signal the first matmuls sooner."""
            w = W[i]
            kp_pss[i] = []
            for c0 in range(0, w, cs):
                n = min(cs, w - c0)
                kt_t = ktp.tile([P, 512], f16, tag="kt",
                                name=f"kt_{i}_{c0}")
                nc.gpsimd.dma_start(kt_t[:, :n],
                                    kT_d[:, offs[i] + c0:
                                         offs[i] + c0 + n])
                kp_ps = pmix.tile([P, 512], f32, tag="mix",
                                  name=f"kp_ps_{i}_{c0}")
                nc.tensor.matmul(kp_ps[:, :n], wk_sb, kt_t[:, :n],
                                 start=True, stop=True)
                kp_pss[i].append((c0, n, kp_ps))

        def emit_load_cast(i, after=None):
            """fp16 kp copies for group i (DVE side).  `after` pins them
            behind an earlier DVE instruction so the static schedule can't
            hoist them in front of adds they would stall."""
            insts = []
            for c0, n, kp_ps in kp_pss[i]:
                ci = nc.vector.tensor_copy(
                    kp_sb[:, offs[i] + c0: offs[i] + c0 + n],
                    kp_ps[:, :n])
                if after is not None:
                    add_dep_helper(ci.ins, after.ins,
                                   reason="kp cast after earlier adds")
                insts.append(ci)
            return insts

        def emit_mask(i):
            # K=2 rank-2 init: rows in the band get the valid-len mask, rows
            # outside it get MASK_VAL so they exp to exactly 0 later (the
            # probs->pt reduction matmul sums over all four bands).
            w = W[i]
            # pad to whole PSUM banks (512 f32) so 512-col matmul chunks
            # never cross a bank boundary
            wpad = ((w + 511) // 512) * 512
            sc = scp.tile([P, wpad], f32, tag="sc", name=f"scores_{i}")
            scores[i] = sc
            scpads[i] = wpad
            for c0 in range(0, w, 512):
                c1 = min(c0 + 512, w)
                nc.tensor.matmul(
                    sc[:, c0:c1],
                    ind_sb[0:2, i * P:(i + 1) * P],
                    mask_sb[0:2, i * Wmax + c0: i * Wmax + c1],
                    start=True, stop=l0flags[i] and c1 == w,
                    skip_group_check=True)

        def emit_score_mms(i, feats, r0, r1, c_lo, c_hi):
            """one-hot [128,32] wv-window matmuls accumulating rows
            r0..r1 of group i (cols c_lo:c_hi) into the group's band."""
            w = W[i]
            sc = scores[i]
            band = slice(GROUP_ROWS * i, GROUP_ROWS * (i + 1))
            for j in range(r1 - r0):
                s = r0 + j
                last_row = s == GROUP_ROWS - 1
                for c0 in range(c_lo, c_hi, 512):
                    c1 = min(c0 + 512, c_hi)
                    nc.tensor.matmul(
                        sc[band, c0:c1],
                        wvd_t[:, GROUP_ROWS - 1 - s: 2 * GROUP_ROWS - 1 - s],
                        feats[:, j * w + c0: j * w + c1],
                        start=False,
                        stop=last_row and c1 == c_hi,
                        tile_position=(0, GROUP_ROWS * i),
                        skip_group_check=True)

        def emit_scores(i, ranges):
            """adds + tanh + one-hot score matmuls for group i over the
            given (row0, row1) ranges.  Returns (adds, tanhs)."""
            w = W[i]
            adds = []
            tanhs = []
            for r0, r1 in ranges:
                nr = r1 - r0
                feats = featsp.tile([P, nr * w], f16, tag="feats",
                                    name=f"feats_{i}_{r0}")
                for j in range(nr):
                    s = GROUP_ROWS * i + r0 + j
                    adds.append(nc.vector.tensor_scalar_add(
                        feats[:, j * w:(j + 1) * w],
                        kp_sb[:, offs[i]: offs[i] + w],
                        qp_sb[:, s: s + 1]))
                tanhs.append(nc.scalar.activation(
                    feats[:], feats[:],
                    mybir.ActivationFunctionType.Tanh))
                emit_score_mms(i, feats, r0, r1, 0, w)
            return adds, tanhs

        def emit_vdma(i):
            for c in range(nstrips[i]):
                cw = min(P, W[i] - c * P)
                vts[(i, c)] = const.tile([P, VS], f16, name=f"v_{i}_{c}")
                nc.gpsimd.dma_start(vts[(i, c)][:cw, :],
                                    V_d[i, c * P: c * P + cw, :])

        def emit_rmax(i):
            # per-group -max; 0 outside the band so those rows (scores
            # MASK_VAL) exp to exactly 0.
            sc = scores[i]
            band = slice(GROUP_ROWS * i, GROUP_ROWS * (i + 1))
            nrm = small.tile([P, 1], f32, bufs=2, tag="nrm",
                             name=f"nrm_{i}")
            nrms[i] = nrm
            nc.vector.memset(nrm[:], 0.0)
            nc.vector.reduce_max(nrm[band, :], sc[band, 0:W[i]],
                                 axis=mybir.AxisListType.X, negate=True)

        def emit_softmax_attnv(i):
            w = W[i]
            n = nstrips[i]
            sc = scores[i]
            nrm = nrms[i]
            band = slice(GROUP_ROWS * i, GROUP_ROWS * (i + 1))
            wpad = n * P
            tail = i == B - 1
            probs = probsp.tile([P, wpad], f16, tag="probs",
                                name=f"probs_{i}")
            if wpad > w:
                nc.gpsimd.memset(probs[:, w:], 0.0)
            # one spare column holds the softmax denominators (pt.T @ ones
            # matmuls) -- same tile as pt so no new PSUM-tile serialization
            pt_ps = pmix.tile([P, GROUP_ROWS * n + 1], f32, tag="mix",
                              name=f"pt_ps_{i}")

            def emit_pt(c):
                # "transpose" probs via probs.T @ band-identity: full-height
                # stationary (base partition 0); non-band rows are exactly 0
                # so the cross-band sum picks out the band
                nc.tensor.matmul(pt_ps[:, GROUP_ROWS * c:
                                       GROUP_ROWS * (c + 1)],
                                 probs[:, c * P:(c + 1) * P],
                                 bident[:, 0:GROUP_ROWS],
                                 start=True, stop=True,
                                 skip_group_check=True)

            if tail and w > P:
                # strip-split exp so the first pt matmul overlaps the
                # second exp strip on the tail chain
                nc.scalar.activation(probs[:, 0:P], sc[:, 0:P],
                                     mybir.ActivationFunctionType.Exp,
                                     bias=nrm[:, 0:1], scale=1.0)
                emit_pt(0)
                exp_instrs[i] = nc.scalar.activation(
                    probs[:, P:w], sc[:, P:w],
                    mybir.ActivationFunctionType.Exp,
                    bias=nrm[:, 0:1], scale=1.0)
                for c in range(1, n):
                    emit_pt(c)
            else:
                exp_instrs[i] = nc.scalar.activation(
                    probs[:, :w], sc[:, 0:w],
                    mybir.ActivationFunctionType.Exp,
                    bias=nrm[:, 0:1], scale=1.0)
                for c in range(n):
                    emit_pt(c)
            pt_sb = small.tile([P, GROUP_ROWS * n], f16, tag="pt",
                               name=f"pt_sb_{i}")
            ptc = nc.vector.tensor_copy(pt_sb[:], pt_ps[:, 0:GROUP_ROWS * n])
            # the last group's attnV accumulates into the pad columns of its
            # own (tile-dep-wise already dead) score tile, so it never
            # serializes against the previous group's out_ps2 reads
            scpad = scpads[i]
            if tail and scpad - VS >= w:
                ot, base = scores[i], scpad - VS
            else:
                ot, base = out_ps2, (i % 2) * VS
            # softmax denominators: tiny pt.T @ ones matmuls (PE, off the
            # DVE chain)
            rs_col = GROUP_ROWS * n
            for c in range(n):
                cw = min(P, w - c * P)
                nc.tensor.matmul(
                    pt_ps[band, rs_col:rs_col + 1],
                    pt_sb[:cw, GROUP_ROWS * c: GROUP_ROWS * (c + 1)],
                    bident[:cw, GROUP_ROWS:GROUP_ROWS + 1],
                    start=(c == 0), stop=(c == n - 1),
                    tile_position=(0, GROUP_ROWS * i),
                    skip_group_check=True)
            nc.vector.reciprocal(rinv[band, :],
                                 pt_ps[band, rs_col:rs_col + 1])
            for c in range(n):
                cw = min(P, w - c * P)
                nc.tensor.matmul(
                    ot[band, base:base + VS],
                    pt_sb[:cw, GROUP_ROWS * c: GROUP_ROWS * (c + 1)],
                    vts[(i, c)][:cw, :],
                    start=(c == 0), stop=(c == n - 1),
                    tile_position=(0, GROUP_ROWS * i),
                    skip_group_check=True)
            # normalize + ship.  The last two groups run their scale on the
            # (by then idle) ACT engine via Copy's free affine -- the DVE
            # carries only the pt casts and reciprocals in the tail -- and
            # their DMAs issue from the still-warm scalar queue (sync and
            # gpsimd have been asleep for ~50us and pay a wake penalty
            # right on the retire path).
            if i >= B - 2:
                mulact_instrs[i] = nc.scalar.activation(
                    out_sb[band, :],
                    ot[band, base:base + VS],
                    mybir.ActivationFunctionType.Copy,
                    scale=rinv[band, 0:1])
                # only the LAST group's DMA issues from the scalar queue;
                # the next-to-last one would block the final Copy there
                q = nc.scalar if tail else nc.gpsimd
                q.dma_start(
                    out_d[GROUP_ROWS * i: GROUP_ROWS * (i + 1), :],
                    out_sb[band, :])
            else:
                nc.vector.tensor_scalar_mul(out_sb[band, :],
                                            ot[band, base:base + VS],
                                            rinv[band, 0:1])
                q = nc.sync if i % 2 == 0 else nc.gpsimd
                q.dma_start(out_d[GROUP_ROWS * i: GROUP_ROWS * (i + 1), :],
                            out_sb[band, :])
            return ptc

        # ---- head: project q rows 0:32 (slim blob), kp chunk 1, then the
        # first two rows' tanh straight from PSUM with bias=qp.  The head
        # tanhs read DUPLICATE kp projections: the framework serializes a
        # PSUM tile's DVE reads behind ACT's last read of the same tile, so
        # the fp16 casts get their own copies (PE is idle; the extra matmuls
        # are free) ----
        qp_ps = pmix.tile([P, GROUP_ROWS], f32, tag="mix", name="qp_ps")
        nc.tensor.matmul(qp_ps[:], wq_sb,
                         qt32_sb[:], start=True, stop=True,
                         skip_group_check=True)
        nc.vector.tensor_copy(qp_sb[:, 0:GROUP_ROWS], qp_ps[:])

        w0 = W[0]
        HEAD_ROWS = 3
        kp0_chunks = []
        dup_chunks = []
        if not l0flags[0]:
            # duplicate projections of the first two chunks: the head rows'
            # tanh reads these so the fp16 casts (reading the originals)
            # never serialize behind ACT
            c0, nn, kt_t = kt0_chunks[0]
            dup1 = pmix.tile([P, 512], f32, tag="mix", name="kp_dup_0")
            nc.tensor.matmul(dup1[:, :nn], wk_sb, kt_t[:, :nn],
                             start=True, stop=True)
            dup_chunks.append((c0, nn, dup1))
            if len(kt0_chunks) > 1:
                # the second dup chunk lives in the (still untouched) out
                # PSUM bank; attnV's start=True reset recycles it later
                c0, nn, kt_t = kt0_chunks[1]
                assert nn <= 2 * VS
                nc.tensor.matmul(out_ps2[:, 0:nn], wk_sb, kt_t[:, :nn],
                                 start=True, stop=True,
                                 skip_group_check=True)
                dup_chunks.append((c0, nn, out_ps2))
        for c0, nn, kt_t in kt0_chunks:
            kp_ps = pmix.tile([P, 512], f32, tag="mix",
                              name=f"kp_ps_0_{c0}")
            nc.tensor.matmul(kp_ps[:, :nn], wk_sb, kt_t[:, :nn],
                             start=True, stop=True)
            kp0_chunks.append((c0, nn, kp_ps))
        kp_pss[0] = kp0_chunks

        if not l0flags[0]:
            # first HEAD_ROWS rows of group 0: tanh(kp + qp[s]) via
            # activation bias, reading the duplicate kp PSUM chunks
            # (col-chunked so the first tanh starts as soon as data lands)
            feats0h = featsp.tile([P, HEAD_ROWS * w0], f16, tag="feats",
                                  name="feats_0_0")
            covered = sum(nn for _, nn, _ in dup_chunks)
            head_chunks = dup_chunks + [ch for ch in kp0_chunks
                                        if ch[0] >= covered]
            for j in range(HEAD_ROWS):
                for c0, nn, kp_ps in head_chunks:
                    nc.scalar.activation(
                        feats0h[:, j * w0 + c0: j * w0 + c0 + nn],
                        kp_ps[:, :nn],
                        mybir.ActivationFunctionType.Tanh,
                        bias=qp_sb[:, j: j + 1], scale=1.0)
            # kp casts for the rest of group 0's rows (from the originals,
            # which no ACT instruction ever reads)
            g0_casts = []
            for c0, nn, kp_ps in kp0_chunks:
                g0_casts.append(nc.vector.tensor_copy(
                    kp_sb[:, c0:c0 + nn], kp_ps[:, :nn]))
            emit_mask(0)
            emit_score_mms(0, feats0h, 0, HEAD_ROWS, 0, w0)
            adds48, _ = emit_scores(0, [(3, 5), (5, 9)])
            qp96_ps = pmix.tile([P, P - GROUP_ROWS], f32, tag="mix",
                                name="qp96_ps")
            nc.tensor.matmul(qp96_ps[:], wq_sb,
                             blobq[:], start=True, stop=True,
                             skip_group_check=True)
            qp96c = nc.vector.tensor_copy(qp_sb[:, GROUP_ROWS:],
                                          qp96_ps[:])
            # keep the head DVE chain in order: the g0 casts feed the first
            # adds; nothing may be scheduled in front of them
            add_dep_helper(qp96c.ins, g0_casts[-1].ins,
                           reason="qp rest copy after g0 kp casts")
            emit_load_mm(1)
            emit_scores(0, [(9, 17)])
            emit_load_mm(2)
            emit_load_mm(3)
            emit_scores(0, [(17, 32)])
        else:
            for c0, nn, kp_ps in kp0_chunks:
                nc.vector.tensor_copy(kp_sb[:, c0:c0 + nn], kp_ps[:, :nn])
            emit_mask(0)
            qp96_ps = pmix.tile([P, P - GROUP_ROWS], f32, tag="mix",
                                name="qp96_ps")
            nc.tensor.matmul(qp96_ps[:], wq_sb,
                             blobq[:], start=True, stop=True,
                             skip_group_check=True)
            nc.vector.tensor_copy(qp_sb[:, GROUP_ROWS:],
                                  qp96_ps[:])
            emit_load_mm(1)
            emit_load_mm(2)
            emit_load_mm(3)
            adds48 = None
        emit_load_cast(1, after=adds48[-1] if adds48 else None)
        emit_vdma(0)

        # ---- main loop: group i's first tanh batch precedes group i-1's
        # softmax; the last group's reduce_max precedes the exp-gated DVE
        # work of groups B-2/B-1 so the tail chain starts immediately ----
        for i in range(1, B):
            emit_mask(i)
            emit_vdma(i)
            if i + 1 < B:
                emit_load_cast(i + 1)
            if not l0flags[i]:
                if i < B - 1:
                    # split batches keep the PE score-matmul stream fed as
                    # soon as each half's tanh lands
                    emit_scores(i, [(0, 16)])
                    emit_rmax(i - 1)
                    emit_scores(i, [(16, 32)])
                    emit_softmax_attnv(i - 1)
                else:
                    emit_scores(i, [(0, 16)])
                    emit_rmax(i - 1)
                    # fine-grained last batches: the per-row score matmuls
                    # keep pace with the tanh stream and only the final
                    # row's matmul trails the last tanh
                    adds_t, tanhs_t = emit_scores(
                        i, [(16, 22), (22, 28), (28, 31), (31, 32)])
                    emit_rmax(i)
                    ptc_prev = emit_softmax_attnv(i - 1)
                    # the prev group's softmax work must never be scheduled
                    # in front of the tail group's last adds/tanh (DVE/ACT
                    # in-order streams would stall the tail on it)
                    add_dep_helper(ptc_prev.ins, adds_t[-1].ins,
                                   reason="prev pt cast after tail adds")
                    if exp_instrs[i - 1] is not None:
                        add_dep_helper(exp_instrs[i - 1].ins,
                                       tanhs_t[-1].ins,
                                       reason="prev exp after tail tanhs")
                    emit_softmax_attnv(i)
            else:
                emit_rmax(i - 1)
                emit_softmax_attnv(i - 1)
                if i == B - 1:
                    emit_rmax(i)
                    emit_softmax_attnv(i)
        # the next-to-last group's ACT-side out scale must never be
        # scheduled in front of the last group's exps on the ACT queue
        if mulact_instrs[B - 2] is not None and exp_instrs[B - 1] is not None:
            add_dep_helper(mulact_instrs[B - 2].ins,
                           exp_instrs[B - 1].ins,
                           reason="prev out scale after tail exps")

    nc.compile()
    return nc


def _get_program(cfg):
    if cfg not in _prog_cache:
        _prog_cache[cfg] = _build_program(cfg)
    return _prog_cache[cfg]


def _width(L):
    # even-padded computed width; valid_len==0 means "uniform over all KV"
    if L <= 0:
        return KV
    L = min(L, KV)
    return min(KV, max(2, 2 * math.ceil(L / 2)))


def kernel(queries, keys, values, valid_lens, Wq, Wk, wv):
    global LAST_EXEC_NS
    queries = np.ascontiguousarray(np.asarray(queries), dtype=np.float32)
    keys = np.ascontiguousarray(np.asarray(keys), dtype=np.float32)
    values = np.ascontiguousarray(np.asarray(values), dtype=np.float32)
    Wq = np.ascontiguousarray(np.asarray(Wq), dtype=np.float32)
    Wk = np.ascontiguousarray(np.asarray(Wk), dtype=np.float32)
    wv = np.ascontiguousarray(np.asarray(wv), dtype=np.float32)
    vl = [int(x) for x in np.asarray(valid_lens)]

    W_b = [_width(L) for L in vl]
    # widest group first: its long tanh stream gives the DVE adds of every
    # later group enough runway; smallest group last for a short tail
    gorder = sorted(range(B), key=lambda b: (-W_b[b], b))
    Ws = tuple(W_b[b] for b in gorder)
    l0flags = tuple(vl[b] == 0 for b in gorder)
    Wmax = max(Ws)

    nc = _get_program((Ws, l0flags))

    kT = np.concatenate(
        [keys[gorder[i]][:Ws[i]].T for i in range(B)], axis=1)
    kT = np.ascontiguousarray(kT.astype(np.float16))     # [128, SW]
    Vm = np.ascontiguousarray(
        np.stack([values[gorder[i]] for i in range(B)]).astype(np.float16))
    # row 0: band indicator x per-group valid mask; row 1: outside-band
    # indicator x MASK_VAL (so non-band score rows exp to exactly 0)
    ind = np.zeros((2, B * P), np.float16)
    for i in range(B):
        ind[0, i * P + GROUP_ROWS * i: i * P + GROUP_ROWS * (i + 1)] = 1.0
        ind[1, i * P: (i + 1) * P] = 1.0
        ind[1, i * P + GROUP_ROWS * i: i * P + GROUP_ROWS * (i + 1)] = 0.0
    mask = np.zeros((2, B * Wmax), np.float16)
    mask[1, :] = MASK_VAL
    for i in range(B):
        L = vl[gorder[i]]
        if L > 0:
            mask[0, i * Wmax + min(L, Ws[i]): i * Wmax + Ws[i]] = MASK_VAL
    # [128, 63] window: wv at col 31 so window [31-s : 63-s] puts wv at
    # in-band position s
    wvd = np.zeros((P, 2 * GROUP_ROWS - 1), np.float16)
    wvd[:, GROUP_ROWS - 1] = wv.astype(np.float16)
    # band identity + a trailing ones column (softmax denominator matmuls)
    bident = np.ascontiguousarray(np.concatenate(
        [np.tile(np.eye(GROUP_ROWS, dtype=np.float16), (B, 1)),
         np.ones((P, 1), np.float16)], axis=1))

    wk16u8 = np.ascontiguousarray(Wk.astype(np.float16)).view(np.uint8)
    wq16u8 = np.ascontiguousarray(Wq.astype(np.float16)).view(np.uint8)
    shared = {"kT": kT, "V": Vm, "ind": ind,
              "mask": mask, "wvd": wvd, "bident": bident}
    in_maps = []
    for c in range(N_CORES):
        qT = np.concatenate(
            [queries[gorder[i], c * GROUP_ROWS:(c + 1) * GROUP_ROWS, :].T
             for i in range(B)], axis=1).astype(np.float16)
        qt32u8 = np.ascontiguousarray(qT[:, 0:GROUP_ROWS]).view(np.uint8)
        m = dict(shared)
        m["blobq"] = np.ascontiguousarray(qT[:, GROUP_ROWS:])
        m["hblob"] = np.ascontiguousarray(
            np.concatenate([wk16u8, wq16u8, qt32u8], axis=1))
        in_maps.append(m)

    if SIMULATE:
        from concourse.bass_interp import CoreSim
        outs = []
        for c in range(N_CORES):
            sim = CoreSim(nc, trace=False)
            for name, v in in_maps[c].items():
                sim.tensor(name)[:] = v
            sim.simulate(check_with_hw=False)
            outs.append(sim.tensor("out").copy())
    else:
        from concourse import bass_utils
        kw = {}
        if PROFILE:
            kw = {"trace": True}
        res = bass_utils.run_bass_kernel_spmd(nc, in_maps, list(range(N_CORES)),
                                              **kw)
        if PROFILE:
            LAST_EXEC_NS = res.exec_time_ns
            global LAST_RESULTS
            LAST_RESULTS = res
        outs = [res.results[c]["out"] for c in range(N_CORES)]

    out = np.zeros((B, Q, VS), np.float32)
    for c in range(N_CORES):
        for i in range(B):
            out[gorder[i], c * GROUP_ROWS:(c + 1) * GROUP_ROWS, :] = \
                outs[c][GROUP_ROWS * i: GROUP_ROWS * (i + 1), :]
    return out
